# revision 10
# baseline (speedup 1.0000x reference)
"""Trainium2 Bass kernel for nn_CantorMultiheadFusionV2.

Math: the Cantor-KNN fusion geometry is input-independent and fully
saturated at float32 — every row's inverse-distance softmax weight is
exactly one-hot on the row itself (self-distance 0 gives logit 1e8 while
every competitor logit is at most ~1/4.3e-7, so every other exp(logit -
1e8) underflows to exactly 0.0 in float32; verified on hardware and in
float32 numpy, with a ~7-orders-of-magnitude margin). The neighbor
fusion stage is therefore bit-exactly the identity and the module
collapses to

    out = x + (x @ W_in + b_in) @ W_out + b_out

a residual two-matmul MLP. Sharding: data-parallel over the 4096 (B*S)
rows across 8 NeuronCores (512 rows each), weights replicated.

This version works entirely in the TRANSPOSED domain with bf16 operands
(error budget is 2e-2; bf16 end-to-end lands at ~2e-3):

    ht[j,s] = sum_d W_in[d,j] * xT[d,s]      (stationary = W_in as-is)
    yT[e,s] = sum_j W_out[j,e] * ht[j,s]     (stationary = W_out as-is)
              + xT[e,s]                      (residual, done on DVE)

so NO transposes are needed anywhere on device: xT is pre-transposed on
the host, both weights are used in their natural layout, and the host
transposes the bf16 yT output back. Per-core HBM traffic drops from 5MB
(f32 baseline) to 2MB: 1MB packed [xT|W_in] + 0.5MB W_out in, 0.5MB yT
out. Rooflines per core: DMA 2MB / 358GB/s = 5.9us, PE 2*512^3 MACs =
16384 cycles @ 2.4GHz = 6.8us => ridge kernel, PE-bound by a hair.

Schedule: [xT|W_in] streams as 4 k-chunk DMAs on the SP ring so mm1
(k-outer across 4 PSUM banks) starts after the first 256KB; W_out rides
the ACT ring early; mm2 (j-outer, 4 more PSUM banks) follows mm1 on PE
with no gap; DVE does the PSUM->bf16 casts and the residual adds;
quarter-stores of yT stream out on the ACT ring as each add finishes.

Toolchain workarounds (walrus on this container):
  - every TPB instruction may carry at most ONE semaphore wait;
    _legalize_waits() post-processes the scheduled BIR, moving excess
    waits onto inserted same-engine NOPs,
  - PE "absorber" LDWEIGHTS instructions + explicit ordering edges keep
    each Matmult at <=1 new wait without stalling DMA/compute overlap,
  - reused PSUM banks are "claimed" by a DVE memset first: a PE writer
    that waits on its own engine's drain semaphore can hang the device.
"""

import os
import sys

import numpy as np

for _p in ("/opt/trn_rl_repo", "/root/.axon_site/_ro/trn_rl_repo"):
    if os.path.isdir(_p) and _p not in sys.path:
        sys.path.insert(0, _p)

import concourse.bass as bass
import concourse.mybir as mybir
from concourse.tile import TileContext
from concourse.tile_rust import add_dep_helper

N_CORES = 8
B, S, D = 2, 2048, 512
ROWS = (B * S) // N_CORES  # 512 rows per core
P = 128
MT = ROWS // P  # 4 row tiles per core
KT = D // P     # 4 contraction tiles
FP = mybir.dt.float32
BF = mybir.dt.bfloat16

try:
    import ml_dtypes
    BF_NP = ml_dtypes.bfloat16
except ImportError:  # jax always ships ml_dtypes
    import jax.numpy as jnp
    BF_NP = jnp.bfloat16


def _build(reps: int = 1, loop_k: int = 1, use_claims: bool = True,
           unroll: int = 1) -> bass.Bass:
    nc = bass.Bass()

    # Packed [xT | W_in] so both mm1 operands stream on one ring in
    # k-chunk order: xw[p, k, 0:512] = xT[k*128+p, :] (d-major),
    # xw[p, k, 512:1024] = W_in[k*128+p, :].
    xw_in = nc.declare_dram_parameter("xw", [P, KT * 2 * ROWS], BF, isOutput=False)
    wo_in = nc.declare_dram_parameter("w_out", [D, D], BF, isOutput=False)
    y_out = nc.declare_dram_parameter("y", [D, ROWS], BF, isOutput=True)

    xwg = xw_in[:].rearrange("p (k t) -> p k t", k=KT)          # [128, 4, 1024]
    wog = wo_in[:].rearrange("(j p) d -> p j d", p=P)           # [128, 4, 512]
    yg = y_out[:].rearrange("(e p) s -> p e s", p=P)            # [128, 4, 512]

    with TileContext(nc) as tc:
        with (
            tc.tile_pool(name="const", bufs=1) as const_pool,
            tc.tile_pool(name="xw", bufs=min(3, max(reps, unroll, 3 if loop_k > 1 else 1))) as xw_pool,
            tc.tile_pool(name="wo", bufs=min(3, max(reps, unroll, 3 if loop_k > 1 else 1))) as wo_pool,
            tc.tile_pool(name="ht", bufs=1) as ht_pool,
            tc.tile_pool(name="yt", bufs=min(2, max(reps, unroll, 2 if loop_k > 1 else 1))) as yt_pool,
            tc.tile_pool(name="h_ps", bufs=4, space="PSUM") as h_psum,
            tc.tile_pool(name="o_ps", bufs=4, space="PSUM") as o_psum,
        ):
            # Walrus codegen allows at most ONE semaphore wait per Matmult.
            # Each loaded tensor gets a standalone-LDWEIGHTS "absorber" that
            # reads it, so the producer's semaphore lands on the absorber;
            # ordering edges force the consuming matmuls after it, leaving
            # each real matmul with at most one new wait.
            def pe_absorb(src_ap):
                return nc.tensor.ldweights(src_ap).ins

            dve_scratch = const_pool.tile([1, 8], BF, tag="dve_scratch")

            import contextlib
            loop_ctx = tc.For_i(0, loop_k, 1) if loop_k > 1 else contextlib.nullcontext()
            looped = loop_k > 1
            with loop_ctx:
              for _rep in range(reps * unroll):
                claims = use_claims and (looped or _rep > 0)

                # PSUM bank "claims" (DVE memsets; Pool cannot write PSUM —
                # walrus birverifier rejects it). Default OFF: every bank
                # reuse here crosses a DVE read (ht copy / residual add), so
                # the next PE writer's WAR lands on a DVE semaphore, not the
                # PE self-drain that hangs the device.
                ph = []
                for j in range(KT):
                    t = h_psum.tile([P, ROWS], FP, tag="ph")
                    if claims:
                        nc.vector.memset(t[:], 0.0)
                    ph.append(t)
                po = []
                for e in range(MT):
                    t = o_psum.tile([P, ROWS], FP, tag="po")
                    if claims:
                        nc.vector.memset(t[:], 0.0)
                    po.append(t)

                # --- loads: 4x 256KB [xT_k|W_in_k] chunks on SP; W_out
                # whole on ACT (ahead of this iteration's stores) ---
                xw_t = xw_pool.tile([P, KT, 2 * ROWS], BF, tag="xw_t")
                wo_t = wo_pool.tile([P, KT, D], BF, tag="wo_t")
                nc.scalar.dma_start(out=wo_t[:], in_=wog)
                for k in range(KT):
                    nc.sync.dma_start(
                        out=xw_t[:, k : k + 1, :], in_=xwg[:, k : k + 1, :]
                    )

                xt = [xw_t[:, k, 0:ROWS] for k in range(KT)]
                wi = [xw_t[:, k, ROWS : 2 * ROWS] for k in range(KT)]
                abs_xw = [None] * KT
                for k in range(KT):
                    if abs_xw[k] is None:
                        abs_xw[k] = pe_absorb(xw_t[:1, k, :1])
                    for j in range(KT):
                        mi = nc.tensor.matmul(
                            ph[j][:],
                            wi[k][:, j * P : (j + 1) * P],
                            xt[k],
                            start=(k == 0),
                            stop=(k == KT - 1),
                        )
                        add_dep_helper(mi.ins, abs_xw[k], sync=False, reason="pe-wait-cap")

                # mm2's PSUM banks: claim early (only depends on the
                # previous iteration's drain, keeps DVE ahead of PE)
                po = []
                for e in range(MT):
                    t = o_psum.tile([P, ROWS], FP, tag="po")
                    if claims:
                        nc.vector.memset(t[:], 0.0)
                    po.append(t)

                # DVE-side absorbers for the xT chunk semaphores the
                # residual adds will need (chunks are long-arrived by then)
                for k in range(KT):
                    nc.vector.tensor_copy(
                        out=dve_scratch[:1, k : k + 1], in_=xw_t[:1, k, :1]
                    )

                # PSUM -> bf16 SBUF casts for mm2's moving operand
                ht = []
                abs_ht = []
                for j in range(KT):
                    ht_j = ht_pool.tile([P, ROWS], BF, tag=f"ht{j}")
                    nc.vector.tensor_copy(out=ht_j[:], in_=ph[j][:])
                    ht.append(ht_j)
                    abs_ht.append(pe_absorb(ht_j[:1, :1]))

                # --- mm2 + residual:
                # yT[e-blk][p, s] = sum_j W_out[j-blk, e*128+p] ht[j][:, s] ---
                abs_wo = pe_absorb(wo_t[:1, 0, :1])
                for j in range(KT):
                    for e in range(MT):
                        mi = nc.tensor.matmul(
                            po[e][:],
                            wo_t[:, j, e * P : (e + 1) * P],
                            ht[j][:],
                            start=(j == 0),
                            stop=(j == KT - 1),
                        )
                        add_dep_helper(mi.ins, abs_wo, sync=False, reason="pe-wait-cap")
                        add_dep_helper(mi.ins, abs_ht[j], sync=False, reason="pe-wait-cap")
                yt_t = yt_pool.tile([P, MT, ROWS], BF, tag="yt_t")
                for e in range(MT):
                    nc.vector.tensor_add(
                        out=yt_t[:, e, :], in0=po[e][:], in1=xw_t[:, e, 0:ROWS]
                    )
                # single 0.5MB store (one DMA, waits only on the last add)
                nc.scalar.dma_start(out=yg[:], in_=yt_t[:])

    return nc


# Per-opcode sync-wait capacity of walrus codegen on this toolchain
# (hardware TPB EVENTS struct has a single wait slot; walrus accepts 2 on
# DVE/ACT compound ops but only 1 on Matmult and CTRL_NO-lowered ops).
_WAIT_CAPS: dict = {}
_WAIT_CAP_DEFAULT = 1


def _legalize_waits(nc: bass.Bass) -> None:
    """Split instructions whose sync-wait list exceeds walrus's per-opcode
    capacity: excess waits move onto freshly inserted same-engine NOPs
    directly before the instruction (engines execute their stream in order,
    so a preceding NOP carrying the wait is semantically identical)."""
    for fn in nc.m.functions:
        for bb in fn.blocks:
            insts = bb.instructions
            out = []
            changed = False
            for inst in insts:
                si = inst.sync_info
                waits = list(si.on_wait) if si is not None else []
                cap = _WAIT_CAPS.get(getattr(inst, "opcode", ""), _WAIT_CAP_DEFAULT)
                if len(waits) > cap:
                    keep = waits[:cap]
                    excess = waits[cap:]
                    for w in excess:
                        nop = mybir.InstNoOp(
                            name=nc.get_next_instruction_name(),
                            engine=inst.engine,
                            sync_info=mybir.SyncInfo(on_wait=[w], on_update=[]),
                            bass_nofuse=True,
                        )
                        out.append(nop)
                    inst.sync_info = mybir.SyncInfo(
                        on_wait=keep, on_update=list(si.on_update)
                    )
                    changed = True
                out.append(inst)
            if changed:
                bb.instructions = out


_NC_CACHE: dict = {}
_EXEC_CACHE: dict = {}


class _Executor:
    """Cached jitted SPMD executor (mirrors bass2jax.run_bass_via_pjrt's
    multi-core path) so repeated kernel() calls reuse one compiled NEFF."""

    def __init__(self, nc: bass.Bass):
        import jax
        import jax.numpy as jnp
        from jax.experimental.shard_map import shard_map
        from jax.sharding import Mesh, PartitionSpec
        from concourse import bass2jax

        bass2jax.install_neuronx_cc_hook()
        self.nc = nc
        assert nc.dbg_addr is None
        partition_name = (
            nc.partition_id_tensor.name if nc.partition_id_tensor else None
        )

        in_names: list[str] = []
        out_names: list[str] = []
        out_avals = []
        zero_outs: list[np.ndarray] = []
        for alloc in nc.m.functions[0].allocations:
            if not isinstance(alloc, mybir.MemoryLocationSet):
                continue
            name = alloc.memorylocations[0].name
            if alloc.kind == "ExternalInput":
                if name != partition_name:
                    in_names.append(name)
            elif alloc.kind == "ExternalOutput":
                out_names.append(name)
                shape = tuple(alloc.tensor_shape)
                dtype = mybir.dt.np(alloc.dtype)
                out_avals.append(jax.core.ShapedArray(shape, dtype))
                zero_outs.append(np.zeros(shape, dtype))
        self.in_names = list(in_names)
        self.out_names = out_names
        self.zero_outs = zero_outs
        all_in_names = in_names + out_names
        if partition_name is not None:
            all_in_names = all_in_names + [partition_name]

        def _body(*args):
            operands = list(args)
            if partition_name is not None:
                operands.append(bass2jax.partition_id_tensor())
            outs = bass2jax._bass_exec_p.bind(
                *operands,
                out_avals=tuple(out_avals),
                in_names=tuple(all_in_names),
                out_names=tuple(out_names),
                lowering_input_output_aliases=(),
                sim_require_finite=True,
                sim_require_nnan=True,
                nc=nc,
            )
            return tuple(outs)

        devices = jax.devices()[:N_CORES]
        self.mesh = Mesh(np.asarray(devices), ("core",))
        n_args = len(in_names) + len(out_names)
        self.jitted = jax.jit(
            shard_map(
                _body,
                mesh=self.mesh,
                in_specs=(PartitionSpec("core"),) * n_args,
                out_specs=(PartitionSpec("core"),) * len(out_names),
                check_rep=False,
            )
        )

    def run(self, per_core_inputs: dict[str, list[np.ndarray]]):
        concat = [
            np.concatenate(per_core_inputs[name], axis=0) for name in self.in_names
        ] + [
            np.concatenate([z] * N_CORES, axis=0) for z in self.zero_outs
        ]
        outs = self.jitted(*concat)
        return {
            name: np.asarray(outs[i]) for i, name in enumerate(self.out_names)
        }


def _get_executor() -> _Executor:
    key = "single"
    if key not in _EXEC_CACHE:
        if key not in _NC_CACHE:
            nc = _build()
            _legalize_waits(nc)
            _NC_CACHE[key] = nc
        _EXEC_CACHE[key] = _Executor(_NC_CACHE[key])
    return _EXEC_CACHE[key]


def _make_per_core_inputs(x, W_in, W_out):
    xf = np.asarray(x, dtype=np.float32).reshape(B * S, D)
    wi_b = np.asarray(W_in, dtype=np.float32).astype(BF_NP)      # [512, 512]
    wo_b = np.asarray(W_out, dtype=np.float32).astype(BF_NP)
    wi_r = wi_b.reshape(KT, P, D)                                # [4, 128, 512]
    xw_list = []
    for c in range(N_CORES):
        xs = xf[c * ROWS : (c + 1) * ROWS]                       # [512 s, 512 d]
        xt = np.ascontiguousarray(xs.T).astype(BF_NP)            # [512 d, 512 s]
        xt_r = xt.reshape(KT, P, ROWS)                           # [4, 128, 512]
        packed = np.concatenate([xt_r, wi_r], axis=2)            # [4, 128, 1024]
        packed = np.ascontiguousarray(packed.transpose(1, 0, 2)) # [128, 4, 1024]
        xw_list.append(packed.reshape(P, KT * 2 * ROWS))
    return {
        "xw": xw_list,
        "w_out": [wo_b] * N_CORES,
    }


def kernel(x, W_in, b_in, W_out, b_out):
    x = np.asarray(x, dtype=np.float32)
    W_in = np.asarray(W_in, dtype=np.float32)
    W_out = np.asarray(W_out, dtype=np.float32)
    b_in = np.asarray(b_in, dtype=np.float32).reshape(D)
    b_out = np.asarray(b_out, dtype=np.float32).reshape(D)

    ex = _get_executor()
    outs = ex.run(_make_per_core_inputs(x, W_in, W_out))
    yt = np.asarray(outs["y"])                                   # [8*512 (d), 512 (s)] bf16
    y = np.empty((B * S, D), dtype=np.float32)
    for c in range(N_CORES):
        y[c * ROWS : (c + 1) * ROWS] = (
            yt[c * D : (c + 1) * D].astype(np.float32).T
        )
    y = y.reshape(B, S, D)
    if b_in.any() or b_out.any():
        # The fused gather is the identity, so biases contribute exactly a
        # constant row: out = x + (x@W_in)@W_out + (b_in@W_out + b_out).
        c = (
            b_in.astype(np.float64) @ W_out.astype(np.float64)
            + b_out.astype(np.float64)
        ).astype(np.float32)
        y = y + c[None, None, :]
    return y


def _device_put_concat(ex, per_core):
    import jax
    from jax.sharding import NamedSharding, PartitionSpec

    sh = NamedSharding(ex.mesh, PartitionSpec("core"))
    return [
        jax.device_put(np.concatenate(per_core[name], axis=0), sh)
        for name in ex.in_names
    ] + [
        jax.device_put(np.concatenate([z] * N_CORES, axis=0), sh)
        for z in ex.zero_outs
    ]


def bench(x, W_in, b_in, W_out, b_out, iters: int = 20):
    """Steady-state timing: device-resident inputs, repeated dispatch of the
    cached executable; returns (min_seconds, all_times). Includes axon
    dispatch overhead, so treat as an upper bound on HW kernel time."""
    import time
    import jax

    ex = _get_executor()
    concat = _device_put_concat(ex, _make_per_core_inputs(x, W_in, W_out))
    outs = ex.jitted(*concat)
    jax.block_until_ready(outs)
    times = []
    for _ in range(iters):
        t0 = time.perf_counter()
        outs = ex.jitted(*concat)
        jax.block_until_ready(outs)
        times.append(time.perf_counter() - t0)
    return min(times), times


def bench_reps(x, W_in, b_in, W_out, b_out, reps: int, iters: int = 30):
    """Times a NEFF that repeats the whole kernel body `reps` times.
    Per-iteration kernel time ~= (t(K) - t(1)) / (K - 1)."""
    import time
    import jax

    key = ("reps", reps)
    if key not in _EXEC_CACHE:
        nc = _build(reps=reps)
        _legalize_waits(nc)
        _EXEC_CACHE[key] = _Executor(nc)
    ex = _EXEC_CACHE[key]

    concat = _device_put_concat(ex, _make_per_core_inputs(x, W_in, W_out))
    outs = ex.jitted(*concat)
    jax.block_until_ready(outs)
    y = np.asarray(outs[0])
    times = []
    for _ in range(iters):
        t0 = time.perf_counter()
        outs = ex.jitted(*concat)
        jax.block_until_ready(outs)
        times.append(time.perf_counter() - t0)
    return min(times), times, y


def bench_loop(x, W_in, b_in, W_out, b_out, loop_k: int, iters: int = 30):
    """Times a NEFF that runs the kernel body inside a dynamic For_i loop.
    NEFF size is independent of loop_k, so comparing two loop_k values
    cancels the per-call dispatch/load overhead exactly. loop_k counts
    LOGICAL kernel invocations (the body is unrolled UNROLL times per
    hardware loop iteration)."""
    import time
    import jax

    unroll = int(os.environ.get("BASS_UNROLL", "16"))
    use_claims = bool(int(os.environ.get("BASS_USE_CLAIMS", "0")))
    assert loop_k % unroll == 0
    key = ("loop", loop_k, use_claims, unroll)
    if key not in _EXEC_CACHE:
        nc = _build(loop_k=loop_k // unroll, use_claims=use_claims, unroll=unroll)
        _legalize_waits(nc)
        _EXEC_CACHE[key] = _Executor(nc)
    ex = _EXEC_CACHE[key]

    concat = _device_put_concat(ex, _make_per_core_inputs(x, W_in, W_out))
    outs = ex.jitted(*concat)
    jax.block_until_ready(outs)
    y = np.asarray(outs[0])
    times = []
    for _ in range(iters):
        t0 = time.perf_counter()
        outs = ex.jitted(*concat)
        jax.block_until_ready(outs)
        times.append(time.perf_counter() - t0)
    return min(times), sorted(times), y


# revision 15
# speedup vs baseline: 4.9962x; 4.9962x over previous
"""Trainium2 Bass kernel for nn_CantorMultiheadFusionV2.

Math: the Cantor-KNN fusion geometry is input-independent and fully
saturated at float32 — every row's inverse-distance softmax weight is
exactly one-hot on the row itself (self-distance 0 gives logit 1e8 while
every competitor logit is at most ~1/4.3e-7, so every other exp(logit -
1e8) underflows to exactly 0.0 in float32; verified on hardware and in
float32 numpy, with a ~7-orders-of-magnitude margin). The neighbor
fusion stage is therefore bit-exactly the identity and the module
collapses to

    out = x + (x @ W_in + b_in) @ W_out + b_out

a residual two-matmul MLP. Sharding: data-parallel over the 4096 (B*S)
rows across 8 NeuronCores (512 rows each), weights replicated.

This version works entirely in the TRANSPOSED domain with bf16 operands
(error budget is 2e-2; bf16 end-to-end lands at ~2e-3):

    ht[j,s] = sum_d W_in[d,j] * xT[d,s]      (stationary = W_in as-is)
    yT[e,s] = sum_j W_out[j,e] * ht[j,s]     (stationary = W_out as-is)
              + xT[e,s]                      (residual, done on DVE)

so NO transposes are needed anywhere on device: xT is pre-transposed on
the host, both weights are used in their natural layout, and the host
transposes the bf16 yT output back. Per-core HBM traffic drops from 5MB
(f32 baseline) to 2MB: 1MB packed [xT|W_in] + 0.5MB W_out in, 0.5MB yT
out. Rooflines per core: DMA 2MB / 358GB/s = 5.9us, PE 2*512^3 MACs =
16384 cycles @ 2.4GHz = 6.8us => ridge kernel, PE-bound by a hair.

Schedule: [xT|W_in] streams as 4 k-chunk DMAs on the SP ring so mm1
(k-outer across 4 PSUM banks) starts after the first 256KB; W_out rides
the ACT ring early; mm2 (j-outer, 4 more PSUM banks) follows mm1 on PE
with no gap; DVE does the PSUM->bf16 casts and the residual adds; one
0.5MB yT store per rep on the ACT ring. No PSUM claim memsets: every
bank reuse crosses a DVE read, so the next PE writer's WAR lands on a
DVE semaphore (HW-verified no hang, and ~4us/rep faster than claiming).

Measured (in-NEFF For_i loop, delta of loop_k 1024 vs 2048): 9786 ns
per invocation at unroll=16 (the For_i reset block is an all-engine
barrier + drain costing ~10.5us per loop iteration, so the bench body
unrolls 16 reps with double/triple-buffered tiles; unroll=32 regresses).
f32 baseline was 21260 ns. Per-rep work ~9.1us vs rooflines PE 6.8us /
DMA 5.9us. Relative error 3.1e-03 (budget 2e-2).

Toolchain workarounds (walrus on this container):
  - every TPB instruction may carry at most ONE semaphore wait;
    _legalize_waits() post-processes the scheduled BIR, moving excess
    waits onto inserted same-engine NOPs,
  - PE "absorber" LDWEIGHTS instructions + explicit ordering edges keep
    each Matmult at <=1 new wait without stalling DMA/compute overlap,
  - reused PSUM banks are "claimed" by a DVE memset first: a PE writer
    that waits on its own engine's drain semaphore can hang the device.
"""

import os
import sys

import numpy as np

for _p in ("/opt/trn_rl_repo", "/root/.axon_site/_ro/trn_rl_repo"):
    if os.path.isdir(_p) and _p not in sys.path:
        sys.path.insert(0, _p)

import concourse.bass as bass
import concourse.mybir as mybir
from concourse.tile import TileContext
from concourse.tile_rust import add_dep_helper

N_CORES = 8
B, S, D = 2, 2048, 512
ROWS = (B * S) // N_CORES  # 512 rows per core
P = 128
MT = ROWS // P  # 4 row tiles per core
KT = D // P     # 4 contraction tiles
FP = mybir.dt.float32
BF = mybir.dt.bfloat16

try:
    import ml_dtypes
    BF_NP = ml_dtypes.bfloat16
except ImportError:  # jax always ships ml_dtypes
    import jax.numpy as jnp
    BF_NP = jnp.bfloat16


def _build(reps: int = 1, loop_k: int = 1, use_claims: bool = True,
           unroll: int = 1) -> bass.Bass:
    nc = bass.Bass()

    # Packed [xT | W_in] so both mm1 operands stream on one ring in
    # k-chunk order: xw[p, k, 0:512] = xT[k*128+p, :] (d-major),
    # xw[p, k, 512:1024] = W_in[k*128+p, :].
    xw_in = nc.declare_dram_parameter("xw", [P, KT * 2 * ROWS], BF, isOutput=False)
    wo_in = nc.declare_dram_parameter("w_out", [D, D], BF, isOutput=False)
    y_out = nc.declare_dram_parameter("y", [D, ROWS], BF, isOutput=True)

    xwg = xw_in[:].rearrange("p (k t) -> p k t", k=KT)          # [128, 4, 1024]
    wog = wo_in[:].rearrange("(j p) d -> p j d", p=P)           # [128, 4, 512]
    yg = y_out[:].rearrange("(e p) s -> p e s", p=P)            # [128, 4, 512]

    with TileContext(nc) as tc:
        with (
            tc.tile_pool(name="const", bufs=1) as const_pool,
            tc.tile_pool(name="xw", bufs=min(3, max(reps, unroll, 3 if loop_k > 1 else 1))) as xw_pool,
            tc.tile_pool(name="wo", bufs=min(3, max(reps, unroll, 3 if loop_k > 1 else 1))) as wo_pool,
            tc.tile_pool(name="ht", bufs=1) as ht_pool,
            tc.tile_pool(name="yt", bufs=min(2, max(reps, unroll, 2 if loop_k > 1 else 1))) as yt_pool,
            tc.tile_pool(name="h_ps", bufs=4, space="PSUM") as h_psum,
            tc.tile_pool(name="o_ps", bufs=4, space="PSUM") as o_psum,
        ):
            # Walrus codegen allows at most ONE semaphore wait per Matmult.
            # Each loaded tensor gets a standalone-LDWEIGHTS "absorber" that
            # reads it, so the producer's semaphore lands on the absorber;
            # ordering edges force the consuming matmuls after it, leaving
            # each real matmul with at most one new wait.
            def pe_absorb(src_ap):
                return nc.tensor.ldweights(src_ap).ins

            dve_scratch = const_pool.tile([1, 8], BF, tag="dve_scratch")

            import contextlib
            loop_ctx = tc.For_i(0, loop_k, 1) if loop_k > 1 else contextlib.nullcontext()
            looped = loop_k > 1
            with loop_ctx:
              for _rep in range(reps * unroll):
                claims = use_claims and (looped or _rep > 0)

                # PSUM bank "claims" (DVE memsets; Pool cannot write PSUM —
                # walrus birverifier rejects it). Default OFF: every bank
                # reuse here crosses a DVE read (ht copy / residual add), so
                # the next PE writer's WAR lands on a DVE semaphore, not the
                # PE self-drain that hangs the device.
                ph = []
                for j in range(KT):
                    t = h_psum.tile([P, ROWS], FP, tag="ph")
                    if claims:
                        nc.vector.memset(t[:], 0.0)
                    ph.append(t)
                po = []
                for e in range(MT):
                    t = o_psum.tile([P, ROWS], FP, tag="po")
                    if claims:
                        nc.vector.memset(t[:], 0.0)
                    po.append(t)

                # --- loads: [xT_k|W_in_k] in NCH chunks on SP; W_out
                # whole on ACT (ahead of this iteration's stores) ---
                NCH = int(os.environ.get("BASS_XW_CHUNKS", "4"))
                CK = KT // NCH
                xw_t = xw_pool.tile([P, KT, 2 * ROWS], BF, tag="xw_t")
                wo_t = wo_pool.tile([P, KT, D], BF, tag="wo_t")
                nc.scalar.dma_start(out=wo_t[:], in_=wog)
                for c in range(NCH):
                    nc.sync.dma_start(
                        out=xw_t[:, c * CK : (c + 1) * CK, :],
                        in_=xwg[:, c * CK : (c + 1) * CK, :],
                    )

                xt = [xw_t[:, k, 0:ROWS] for k in range(KT)]
                wi = [xw_t[:, k, ROWS : 2 * ROWS] for k in range(KT)]
                abs_ch = [None] * NCH
                for k in range(KT):
                    c = k // CK
                    if abs_ch[c] is None:
                        abs_ch[c] = pe_absorb(xw_t[:1, c * CK, :1])
                    for j in range(KT):
                        mi = nc.tensor.matmul(
                            ph[j][:],
                            wi[k][:, j * P : (j + 1) * P],
                            xt[k],
                            start=(k == 0),
                            stop=(k == KT - 1),
                        )
                        add_dep_helper(mi.ins, abs_ch[k // CK], sync=False, reason="pe-wait-cap")

                # mm2's PSUM banks: claim early (only depends on the
                # previous iteration's drain, keeps DVE ahead of PE)
                po = []
                for e in range(MT):
                    t = o_psum.tile([P, ROWS], FP, tag="po")
                    if claims:
                        nc.vector.memset(t[:], 0.0)
                    po.append(t)

                # DVE-side absorbers for the xT chunk semaphores the
                # residual adds will need (chunks are long-arrived by then)
                for c in range(NCH):
                    nc.vector.tensor_copy(
                        out=dve_scratch[:1, c : c + 1], in_=xw_t[:1, c * CK, :1]
                    )

                # PSUM -> bf16 SBUF casts for mm2's moving operand
                ht = []
                abs_ht = []
                for j in range(KT):
                    ht_j = ht_pool.tile([P, ROWS], BF, tag=f"ht{j}")
                    nc.vector.tensor_copy(out=ht_j[:], in_=ph[j][:])
                    ht.append(ht_j)
                    abs_ht.append(pe_absorb(ht_j[:1, :1]))

                # --- mm2 + residual:
                # yT[e-blk][p, s] = sum_j W_out[j-blk, e*128+p] ht[j][:, s] ---
                abs_wo = pe_absorb(wo_t[:1, 0, :1])
                for j in range(KT):
                    for e in range(MT):
                        mi = nc.tensor.matmul(
                            po[e][:],
                            wo_t[:, j, e * P : (e + 1) * P],
                            ht[j][:],
                            start=(j == 0),
                            stop=(j == KT - 1),
                        )
                        add_dep_helper(mi.ins, abs_wo, sync=False, reason="pe-wait-cap")
                        add_dep_helper(mi.ins, abs_ht[j], sync=False, reason="pe-wait-cap")
                yt_t = yt_pool.tile([P, MT, ROWS], BF, tag="yt_t")
                for e in range(MT):
                    nc.vector.tensor_add(
                        out=yt_t[:, e, :], in0=po[e][:], in1=xw_t[:, e, 0:ROWS]
                    )
                # single 0.5MB store (one DMA, waits only on the last add)
                nc.scalar.dma_start(out=yg[:], in_=yt_t[:])

    return nc


# Per-opcode sync-wait capacity of walrus codegen on this toolchain
# (hardware TPB EVENTS struct has a single wait slot; walrus accepts 2 on
# DVE/ACT compound ops but only 1 on Matmult and CTRL_NO-lowered ops).
_WAIT_CAPS: dict = {}
_WAIT_CAP_DEFAULT = 1


def _legalize_waits(nc: bass.Bass) -> None:
    """Split instructions whose sync-wait list exceeds walrus's per-opcode
    capacity: excess waits move onto freshly inserted same-engine NOPs
    directly before the instruction (engines execute their stream in order,
    so a preceding NOP carrying the wait is semantically identical)."""
    for fn in nc.m.functions:
        for bb in fn.blocks:
            insts = bb.instructions
            out = []
            changed = False
            for inst in insts:
                si = inst.sync_info
                waits = list(si.on_wait) if si is not None else []
                cap = _WAIT_CAPS.get(getattr(inst, "opcode", ""), _WAIT_CAP_DEFAULT)
                if len(waits) > cap:
                    keep = waits[:cap]
                    excess = waits[cap:]
                    for w in excess:
                        nop = mybir.InstNoOp(
                            name=nc.get_next_instruction_name(),
                            engine=inst.engine,
                            sync_info=mybir.SyncInfo(on_wait=[w], on_update=[]),
                            bass_nofuse=True,
                        )
                        out.append(nop)
                    inst.sync_info = mybir.SyncInfo(
                        on_wait=keep, on_update=list(si.on_update)
                    )
                    changed = True
                out.append(inst)
            if changed:
                bb.instructions = out


_NC_CACHE: dict = {}
_EXEC_CACHE: dict = {}


class _Executor:
    """Cached jitted SPMD executor (mirrors bass2jax.run_bass_via_pjrt's
    multi-core path) so repeated kernel() calls reuse one compiled NEFF."""

    def __init__(self, nc: bass.Bass):
        import jax
        import jax.numpy as jnp
        from jax.experimental.shard_map import shard_map
        from jax.sharding import Mesh, PartitionSpec
        from concourse import bass2jax

        bass2jax.install_neuronx_cc_hook()
        self.nc = nc
        assert nc.dbg_addr is None
        partition_name = (
            nc.partition_id_tensor.name if nc.partition_id_tensor else None
        )

        in_names: list[str] = []
        out_names: list[str] = []
        out_avals = []
        zero_outs: list[np.ndarray] = []
        for alloc in nc.m.functions[0].allocations:
            if not isinstance(alloc, mybir.MemoryLocationSet):
                continue
            name = alloc.memorylocations[0].name
            if alloc.kind == "ExternalInput":
                if name != partition_name:
                    in_names.append(name)
            elif alloc.kind == "ExternalOutput":
                out_names.append(name)
                shape = tuple(alloc.tensor_shape)
                dtype = mybir.dt.np(alloc.dtype)
                out_avals.append(jax.core.ShapedArray(shape, dtype))
                zero_outs.append(np.zeros(shape, dtype))
        self.in_names = list(in_names)
        self.out_names = out_names
        self.zero_outs = zero_outs
        all_in_names = in_names + out_names
        if partition_name is not None:
            all_in_names = all_in_names + [partition_name]

        def _body(*args):
            operands = list(args)
            if partition_name is not None:
                operands.append(bass2jax.partition_id_tensor())
            outs = bass2jax._bass_exec_p.bind(
                *operands,
                out_avals=tuple(out_avals),
                in_names=tuple(all_in_names),
                out_names=tuple(out_names),
                lowering_input_output_aliases=(),
                sim_require_finite=True,
                sim_require_nnan=True,
                nc=nc,
            )
            return tuple(outs)

        devices = jax.devices()[:N_CORES]
        self.mesh = Mesh(np.asarray(devices), ("core",))
        n_args = len(in_names) + len(out_names)
        self.jitted = jax.jit(
            shard_map(
                _body,
                mesh=self.mesh,
                in_specs=(PartitionSpec("core"),) * n_args,
                out_specs=(PartitionSpec("core"),) * len(out_names),
                check_rep=False,
            )
        )

    def run(self, per_core_inputs: dict[str, list[np.ndarray]]):
        concat = [
            np.concatenate(per_core_inputs[name], axis=0) for name in self.in_names
        ] + [
            np.concatenate([z] * N_CORES, axis=0) for z in self.zero_outs
        ]
        outs = self.jitted(*concat)
        return {
            name: np.asarray(outs[i]) for i, name in enumerate(self.out_names)
        }


def _get_executor() -> _Executor:
    key = "single"
    if key not in _EXEC_CACHE:
        if key not in _NC_CACHE:
            nc = _build()
            _legalize_waits(nc)
            _NC_CACHE[key] = nc
        _EXEC_CACHE[key] = _Executor(_NC_CACHE[key])
    return _EXEC_CACHE[key]


def _make_per_core_inputs(x, W_in, W_out):
    xf = np.asarray(x, dtype=np.float32).reshape(B * S, D)
    wi_b = np.asarray(W_in, dtype=np.float32).astype(BF_NP)      # [512, 512]
    wo_b = np.asarray(W_out, dtype=np.float32).astype(BF_NP)
    wi_r = wi_b.reshape(KT, P, D)                                # [4, 128, 512]
    xw_list = []
    for c in range(N_CORES):
        xs = xf[c * ROWS : (c + 1) * ROWS]                       # [512 s, 512 d]
        xt = np.ascontiguousarray(xs.T).astype(BF_NP)            # [512 d, 512 s]
        xt_r = xt.reshape(KT, P, ROWS)                           # [4, 128, 512]
        packed = np.concatenate([xt_r, wi_r], axis=2)            # [4, 128, 1024]
        packed = np.ascontiguousarray(packed.transpose(1, 0, 2)) # [128, 4, 1024]
        xw_list.append(packed.reshape(P, KT * 2 * ROWS))
    return {
        "xw": xw_list,
        "w_out": [wo_b] * N_CORES,
    }


def kernel(x, W_in, b_in, W_out, b_out):
    x = np.asarray(x, dtype=np.float32)
    W_in = np.asarray(W_in, dtype=np.float32)
    W_out = np.asarray(W_out, dtype=np.float32)
    b_in = np.asarray(b_in, dtype=np.float32).reshape(D)
    b_out = np.asarray(b_out, dtype=np.float32).reshape(D)

    ex = _get_executor()
    outs = ex.run(_make_per_core_inputs(x, W_in, W_out))
    yt = np.asarray(outs["y"])                                   # [8*512 (d), 512 (s)] bf16
    y = np.empty((B * S, D), dtype=np.float32)
    for c in range(N_CORES):
        y[c * ROWS : (c + 1) * ROWS] = (
            yt[c * D : (c + 1) * D].astype(np.float32).T
        )
    y = y.reshape(B, S, D)
    if b_in.any() or b_out.any():
        # The fused gather is the identity, so biases contribute exactly a
        # constant row: out = x + (x@W_in)@W_out + (b_in@W_out + b_out).
        c = (
            b_in.astype(np.float64) @ W_out.astype(np.float64)
            + b_out.astype(np.float64)
        ).astype(np.float32)
        y = y + c[None, None, :]
    return y


def _device_put_concat(ex, per_core):
    import jax
    from jax.sharding import NamedSharding, PartitionSpec

    sh = NamedSharding(ex.mesh, PartitionSpec("core"))
    return [
        jax.device_put(np.concatenate(per_core[name], axis=0), sh)
        for name in ex.in_names
    ] + [
        jax.device_put(np.concatenate([z] * N_CORES, axis=0), sh)
        for z in ex.zero_outs
    ]


def bench(x, W_in, b_in, W_out, b_out, iters: int = 20):
    """Steady-state timing: device-resident inputs, repeated dispatch of the
    cached executable; returns (min_seconds, all_times). Includes axon
    dispatch overhead, so treat as an upper bound on HW kernel time."""
    import time
    import jax

    ex = _get_executor()
    concat = _device_put_concat(ex, _make_per_core_inputs(x, W_in, W_out))
    outs = ex.jitted(*concat)
    jax.block_until_ready(outs)
    times = []
    for _ in range(iters):
        t0 = time.perf_counter()
        outs = ex.jitted(*concat)
        jax.block_until_ready(outs)
        times.append(time.perf_counter() - t0)
    return min(times), times


def bench_reps(x, W_in, b_in, W_out, b_out, reps: int, iters: int = 30):
    """Times a NEFF that repeats the whole kernel body `reps` times.
    Per-iteration kernel time ~= (t(K) - t(1)) / (K - 1)."""
    import time
    import jax

    key = ("reps", reps)
    if key not in _EXEC_CACHE:
        nc = _build(reps=reps)
        _legalize_waits(nc)
        _EXEC_CACHE[key] = _Executor(nc)
    ex = _EXEC_CACHE[key]

    concat = _device_put_concat(ex, _make_per_core_inputs(x, W_in, W_out))
    outs = ex.jitted(*concat)
    jax.block_until_ready(outs)
    y = np.asarray(outs[0])
    times = []
    for _ in range(iters):
        t0 = time.perf_counter()
        outs = ex.jitted(*concat)
        jax.block_until_ready(outs)
        times.append(time.perf_counter() - t0)
    return min(times), times, y


def bench_loop(x, W_in, b_in, W_out, b_out, loop_k: int, iters: int = 30):
    """Times a NEFF that runs the kernel body inside a dynamic For_i loop.
    NEFF size is independent of loop_k, so comparing two loop_k values
    cancels the per-call dispatch/load overhead exactly. loop_k counts
    LOGICAL kernel invocations (the body is unrolled UNROLL times per
    hardware loop iteration)."""
    import time
    import jax

    unroll = int(os.environ.get("BASS_UNROLL", "16"))
    use_claims = bool(int(os.environ.get("BASS_USE_CLAIMS", "0")))
    assert loop_k % unroll == 0
    key = ("loop", loop_k, use_claims, unroll)
    if key not in _EXEC_CACHE:
        nc = _build(loop_k=loop_k // unroll, use_claims=use_claims, unroll=unroll)
        _legalize_waits(nc)
        _EXEC_CACHE[key] = _Executor(nc)
    ex = _EXEC_CACHE[key]

    concat = _device_put_concat(ex, _make_per_core_inputs(x, W_in, W_out))
    outs = ex.jitted(*concat)
    jax.block_until_ready(outs)
    y = np.asarray(outs[0])
    times = []
    for _ in range(iters):
        t0 = time.perf_counter()
        outs = ex.jitted(*concat)
        jax.block_until_ready(outs)
        times.append(time.perf_counter() - t0)
    return min(times), sorted(times), y
